# revision 26
# baseline (speedup 1.0000x reference)
"""Trainium2 Bass kernel for nn_Evaluate_14963666059433 (retrieval_knn).

Per batch element (one per NeuronCore, 8 total):
  - full correlation C = L^T R  ([4096 pixels x 4096 targets], fp32 PE matmul,
    computed in 32 pixel-tiles of [128 x 4096])
  - each C tile: PSUM -> SBUF (ACT) -> DRAM (DMA) -> indirect-DMA gather of the
    24 candidate scalars per pixel (per-partition 4-byte descriptors, offsets
    precomputed on host from offset_x/offset_y)
  - batched DVE/ACT softmax with XLA-CPU-exact underflow semantics
    (prob > 0  iff  exp(y) >= Z * 2^-126, evaluated as exp(y + 126*ln2 - 4) >= Z*e^-4)
  - top-8 selection by a composite key (z ? cost + B - eps*n : -n) via the DVE
    InstMax sorting-network op; payload extraction by key-equality masks.

Host side only marshals inputs (layout/transpose/index arithmetic) and
unshards outputs.
"""

import numpy as np

B, C, NUM, H, W = 8, 256, 24, 64, 64
HW = H * W                      # 4096
K = NUM // 3                    # 8
NT = HW // 128                  # 32 pixel tiles
NBANK = 8
BANKW = 512

# key-construction constants (see analysis: exact vs jax.lax.top_k semantics)
B_KEY = 20000.0
EPS_KEY = 0.004
C4 = float(126 * np.log(2.0) - 4.0)          # 83.33654475055311
THR_SCALE = float(np.exp(-4.0))              # e^-4

_CACHE = {}

# matmul implementation: "f32" (native fp32, 4 cyc/row) or "f16x2"
# (fp16 hi/lo split: C = hh + 2^-11*(hi*lo + lo*hi), 3 passes at 1 cyc/row)
MM_MODE = "f32"
# gather implementation: "unit" (one instr/tile, 1-elem runs) or
# "rows" (NUM instrs/tile of [128,1] row-descriptors; slower, proven HW semantics)
GATHER_MODE = "rows"
# host-side permutation applied to the offset tensor in "unit" mode
# (identity unless HW consumes offsets in a transposed order)
UNIT_PERM = "natural"
LO_SCALE = 2048.0        # lo tensors stored as (x - hi) * 2^11 in fp16
LO_INV = 1.0 / 2048.0


def build_program(mode=None):
    """Build and schedule the (SPMD, per-core) Bass program once."""
    import concourse.bacc as bacc
    import concourse.mybir as mybir
    import concourse.tile as tile
    from concourse import bass

    if mode is None:
        mode = MM_MODE
    f32 = mybir.dt.float32
    f16 = mybir.dt.float16
    i32 = mybir.dt.int32

    nc = bacc.Bacc("TRN2", target_bir_lowering=False, debug=False, num_devices=8)

    if mode == "f32":
        L_d = nc.dram_tensor("lft", [128, 2, HW], f32, kind="ExternalInput")
        R_d = nc.dram_tensor("rgt", [128, 2, HW], f32, kind="ExternalInput")
    else:
        LH_d = nc.dram_tensor("lfh", [128, 2, HW], f16, kind="ExternalInput")
        LL_d = nc.dram_tensor("lfl", [128, 2, HW], f16, kind="ExternalInput")
        RH_d = nc.dram_tensor("rgh", [128, 2, HW], f16, kind="ExternalInput")
        RL_d = nc.dram_tensor("rgl", [128, 2, HW], f16, kind="ExternalInput")
    OX_d = nc.dram_tensor("ofx", [128, NT, NUM], f32, kind="ExternalInput")
    OY_d = nc.dram_tensor("ofy", [128, NT, NUM], f32, kind="ExternalInput")
    GO_d = nc.dram_tensor("gof", [128, NT, NUM], i32, kind="ExternalInput")
    ROWB_d = nc.dram_tensor("rwb", [128, NUM], f32, kind="ExternalInput")
    NEGN_d = nc.dram_tensor("ngn", [128, NUM], f32, kind="ExternalInput")

    CORR_d = nc.dram_tensor("corr_o", [128, NT, K], f32, kind="ExternalOutput")
    OFX_d = nc.dram_tensor("ofx_o", [128, NT, K], f32, kind="ExternalOutput")
    OFY_d = nc.dram_tensor("ofy_o", [128, NT, K], f32, kind="ExternalOutput")

    from contextlib import ExitStack

    with tile.TileContext(nc) as tc, ExitStack() as ctx:
        wpool = ctx.enter_context(tc.tile_pool(name="weights", bufs=1))
        spool = ctx.enter_context(tc.tile_pool(name="stage", bufs=3))
        ppool = ctx.enter_context(tc.tile_pool(name="psum", bufs=1, space="PSUM"))
        dpool = ctx.enter_context(tc.tile_pool(name="cdram", bufs=3, space="DRAM"))
        kpool = ctx.enter_context(tc.tile_pool(name="post", bufs=1))

        if mode == "f32":
            L_sb = wpool.tile([128, 2, HW], f32)
            R_sb = wpool.tile([128, 2, HW], f32)
            nc.sync.dma_start(L_sb[:], L_d[:])
            nc.sync.dma_start(R_sb[:], R_d[:])
        else:
            LH_sb = wpool.tile([128, 2, HW], f16)
            LL_sb = wpool.tile([128, 2, HW], f16)
            RH_sb = wpool.tile([128, 2, HW], f16)
            RL_sb = wpool.tile([128, 2, HW], f16)
            nc.sync.dma_start(LH_sb[:], LH_d[:])
            nc.sync.dma_start(LL_sb[:], LL_d[:])
            nc.sync.dma_start(RH_sb[:], RH_d[:])
            nc.sync.dma_start(RL_sb[:], RL_d[:])
        OX_sb = wpool.tile([128, NT, NUM], f32)
        OY_sb = wpool.tile([128, NT, NUM], f32)
        GO_sb = wpool.tile([128, NT, NUM], i32)
        ROWB_sb = wpool.tile([128, NUM], f32)
        NEGN_sb = wpool.tile([128, NUM], f32)
        cost_all = wpool.tile([128, NT, NUM], f32)

        nc.sync.dma_start(OX_sb[:], OX_d[:])
        nc.sync.dma_start(OY_sb[:], OY_d[:])
        nc.sync.dma_start(GO_sb[:], GO_d[:])
        nc.sync.dma_start(ROWB_sb[:], ROWB_d[:])
        nc.sync.dma_start(NEGN_sb[:], NEGN_d[:])

        HALFW = HW // 2
        for t in range(NT):
            stage = spool.tile([128, HW], f32)
            cdram = dpool.tile([128, HW], f32)
            if mode == "f32":
                cps = ppool.tile([128, HW], f32)
                for h in (0, 1):
                    for j in range(NBANK):
                        nc.tensor.matmul(
                            cps[:, j * BANKW:(j + 1) * BANKW],
                            lhsT=L_sb[:, h, t * 128:(t + 1) * 128],
                            rhs=R_sb[:, h, j * BANKW:(j + 1) * BANKW],
                            start=(h == 0),
                            stop=(h == 1),
                        )
                for j in range(NBANK):
                    # PSUM -> SBUF copy with the 1/TEMPERATURE scaling folded in
                    nc.scalar.mul(
                        out=stage[:, j * BANKW:(j + 1) * BANKW],
                        in_=cps[:, j * BANKW:(j + 1) * BANKW],
                        mul=100.0,
                    )
            else:
                # L already carries the x100 scale; C = hh + 2^-11*(hi*lo + lo*hi)
                ts = slice(t * 128, (t + 1) * 128)
                for half in (0, 1):
                    A = ppool.tile([128, HALFW], f32, tag="psA")
                    Bp = ppool.tile([128, HALFW], f32, tag="psB")
                    passes = [
                        (A, LH_sb, 0, RH_sb, True, False),
                        (A, LH_sb, 1, RH_sb, False, True),
                        (Bp, LH_sb, 0, RL_sb, True, False),
                        (Bp, LH_sb, 1, RL_sb, False, False),
                        (Bp, LL_sb, 0, RH_sb, False, False),
                        (Bp, LL_sb, 1, RH_sb, False, True),
                    ]
                    for tgt, wsb, h, rsb, st, sp in passes:
                        for j in range(4):
                            cols = half * HALFW + j * BANKW
                            nc.tensor.matmul(
                                tgt[:, j * BANKW:(j + 1) * BANKW],
                                lhsT=wsb[:, h, ts],
                                rhs=rsb[:, h, cols:cols + BANKW],
                                start=st,
                                stop=sp,
                            )
                    nc.vector.scalar_tensor_tensor(
                        out=stage[:, half * HALFW:(half + 1) * HALFW],
                        in0=Bp[:],
                        scalar=LO_INV,
                        in1=A[:],
                        op0=mybir.AluOpType.mult,
                        op1=mybir.AluOpType.add,
                    )
            nc.sync.dma_start(cdram[:], stage[:])
            if True:  # "rows": one [128,1] row-descriptor instruction per candidate
                for n in range(NUM):
                    nc.gpsimd.indirect_dma_start(
                        out=cost_all[:, t, n:n + 1],
                        out_offset=None,
                        in_=cdram[:],
                        in_offset=bass.IndirectOffsetOnAxis(
                            ap=GO_sb[:, t, n:n + 1], axis=1
                        ),
                    )

        # ---- batched softmax / key / top-8 ----
        Exp = mybir.ActivationFunctionType.Exp
        mx = kpool.tile([128, NT], f32)
        y = kpool.tile([128, NT, NUM], f32)
        u = kpool.tile([128, NT, NUM], f32)
        us = kpool.tile([128, NT, NUM], f32)
        Z = kpool.tile([128, NT], f32)
        rz = kpool.tile([128, NT], f32)
        prob = kpool.tile([128, NT, NUM], f32)
        thr = kpool.tile([128, NT], f32)
        zf = kpool.tile([128, NT, NUM], mybir.dt.uint8)
        t1 = kpool.tile([128, NT, NUM], f32)
        key = kpool.tile([128, NT, NUM], f32)
        max8 = kpool.tile([128, NT, K], f32)
        mask = kpool.tile([128, NT, K, NUM], f32)
        corr_sel = kpool.tile([128, NT, K], f32)
        ofx_sel = kpool.tile([128, NT, K], f32)
        ofy_sel = kpool.tile([128, NT, K], f32)

        AX = mybir.AxisListType.X
        AOP = mybir.AluOpType

        def seg_bcast(ap2d):      # [128, NT] -> [128, NT, NUM] (step-0 on NUM)
            return ap2d.unsqueeze(2).broadcast_to([128, NT, NUM])

        def row_bcast(ap2d):      # [128, NUM] -> [128, NT, NUM] (step-0 on NT)
            return ap2d.unsqueeze(1).broadcast_to([128, NT, NUM])

        nc.vector.tensor_reduce(out=mx[:], in_=cost_all[:], axis=AX, op=AOP.max)
        nc.vector.tensor_tensor(
            out=y[:], in0=cost_all[:], in1=seg_bcast(mx[:]), op=AOP.subtract
        )
        c4_sb = kpool.tile([128, 1], f32)
        nc.vector.memset(c4_sb[:], C4)
        nc.scalar.activation(out=u[:], in_=y[:], func=Exp)
        nc.scalar.activation(out=us[:], in_=y[:], func=Exp, bias=c4_sb[:])
        nc.vector.tensor_reduce(out=Z[:], in_=u[:], axis=AX, op=AOP.add)
        nc.vector.reciprocal(out=rz[:], in_=Z[:])
        nc.vector.tensor_tensor(
            out=prob[:], in0=u[:], in1=seg_bcast(rz[:]), op=AOP.mult
        )
        nc.vector.tensor_scalar_mul(thr[:], Z[:], THR_SCALE)
        nc.vector.tensor_tensor(
            out=zf[:], in0=us[:], in1=seg_bcast(thr[:]), op=AOP.is_ge
        )
        nc.vector.tensor_tensor(
            out=t1[:], in0=cost_all[:], in1=row_bcast(ROWB_sb[:]), op=AOP.add
        )
        nc.vector.select(
            out=key[:], mask=zf[:], on_true=t1[:], on_false=row_bcast(NEGN_sb[:])
        )
        for t in range(NT):
            nc.vector.max(out=max8[:, t, :], in_=key[:, t, :])
        nc.vector.tensor_tensor(
            out=mask[:],
            in0=key[:].unsqueeze(2).broadcast_to([128, NT, K, NUM]),
            in1=max8[:].unsqueeze(3).broadcast_to([128, NT, K, NUM]),
            op=AOP.is_equal,
        )
        mtmp = kpool.tile([128, NT, K, NUM], f32)
        for sel, payload in (
            (corr_sel, prob),
            (ofx_sel, OX_sb),
            (ofy_sel, OY_sb),
        ):
            nc.vector.tensor_tensor(
                out=mtmp[:],
                in0=mask[:],
                in1=payload[:].unsqueeze(2).broadcast_to([128, NT, K, NUM]),
                op=AOP.mult,
            )
            nc.vector.tensor_reduce(out=sel[:], in_=mtmp[:], axis=AX, op=AOP.add)
        nc.sync.dma_start(CORR_d[:], corr_sel[:])
        nc.sync.dma_start(OFX_d[:], ofx_sel[:])
        nc.sync.dma_start(OFY_d[:], ofy_sel[:])

    nc.compile()
    return nc


def make_in_maps(left_features, right_features, offset_x, offset_y):
    """Marshal full inputs into 8 per-core input maps."""
    left_features = np.ascontiguousarray(left_features, dtype=np.float32)
    right_features = np.ascontiguousarray(right_features, dtype=np.float32)
    offset_x = np.asarray(offset_x, dtype=np.int32)
    offset_y = np.asarray(offset_y, dtype=np.int32)

    rowb = np.broadcast_to(
        (B_KEY - EPS_KEY * np.arange(NUM, dtype=np.float32))[None, :], (128, NUM)
    ).copy()
    negn = np.broadcast_to(
        (-np.arange(NUM, dtype=np.float32))[None, :], (128, NUM)
    ).copy()

    p = np.arange(HW, dtype=np.int64)
    in_maps = []
    for b in range(B):
        lf = left_features[b]                          # [256, 4096]
        rf = right_features[:, b * HW:(b + 1) * HW]    # [256, 4096]
        L_in = np.ascontiguousarray(
            lf.reshape(2, 128, HW).transpose(1, 0, 2)
        )                                              # [128, 2, 4096]
        R_in = np.ascontiguousarray(
            rf.reshape(2, 128, HW).transpose(1, 0, 2)
        )
        if MM_MODE != "f32":
            Ls = L_in * np.float32(100.0)              # fold 1/TEMPERATURE into L
            Lh = Ls.astype(np.float16)
            Ll = ((Ls - Lh.astype(np.float32)) * np.float32(LO_SCALE)).astype(
                np.float16
            )
            Rh = R_in.astype(np.float16)
            Rl = ((R_in - Rh.astype(np.float32)) * np.float32(LO_SCALE)).astype(
                np.float16
            )
        oxT = offset_x[b].reshape(NUM, HW).T           # [4096, 24]
        oyT = offset_y[b].reshape(NUM, HW).T
        # inds[p, n] = p + 64*ox + oy  (== reference offset_to_inds)
        indsT = p[:, None] + 64 * oxT.astype(np.int64) + oyT.astype(np.int64)
        assert indsT.min() >= 0 and indsT.max() < HW, "invalid candidate index"
        # gather offset within a staged C tile [128, 4096]: pi*4096 + q
        gof = (
            np.arange(128, dtype=np.int64)[:, None, None] * HW
            + indsT.reshape(NT, 128, NUM).transpose(1, 0, 2)
        ).astype(np.int32)                             # [128, NT, NUM]
        ox_in = np.ascontiguousarray(
            oxT.astype(np.float32).reshape(NT, 128, NUM).transpose(1, 0, 2)
        )
        oy_in = np.ascontiguousarray(
            oyT.astype(np.float32).reshape(NT, 128, NUM).transpose(1, 0, 2)
        )
        m = {
            "ofx": ox_in,
            "ofy": oy_in,
            "gof": np.ascontiguousarray(gof),
            "rwb": rowb,
            "ngn": negn,
        }
        if MM_MODE == "f32":
            m["lft"] = L_in
            m["rgt"] = R_in
        else:
            m.update({"lfh": Lh, "lfl": Ll, "rgh": Rh, "rgl": Rl})
        in_maps.append(m)
    return in_maps


def unshard(results):
    """results: list of 8 dicts with corr_o/ofx_o/ofy_o [128, NT, K]."""
    off_x = np.empty((B, K, H, W), np.float32)
    off_y = np.empty((B, K, H, W), np.float32)
    corr = np.empty((B, K, HW), np.float32)

    for b, r in enumerate(results):
        # [pi, t, k] -> [k, 128*t + pi]
        def tk(a):
            return np.ascontiguousarray(
                a.reshape(128, NT, K).transpose(2, 1, 0).reshape(K, HW)
            )

        off_x[b] = tk(r["ofx_o"]).reshape(K, H, W)
        off_y[b] = tk(r["ofy_o"]).reshape(K, H, W)
        corr[b] = tk(r["corr_o"])
    return off_x, off_y, corr


def kernel(left_features, right_features, offset_x, offset_y):
    from concourse import bass_utils

    if "nc" not in _CACHE:
        _CACHE["nc"] = build_program()
    nc = _CACHE["nc"]
    in_maps = make_in_maps(left_features, right_features, offset_x, offset_y)
    res = bass_utils.run_bass_kernel_spmd(nc, in_maps, core_ids=list(range(B)))
    return unshard(res.results)
